# revision 18
# baseline (speedup 1.0000x reference)
"""Trainium2 Bass kernel for nn_CustomModel_30451318129113 (GCLSTM+GCNConv GNN).

Math notes (verified against the reference):
- The GCLSTM cell is called with H = C = 0, so every ChebConv sees an
  all-zero input and contributes only its bias => the Chebyshev/edge_weight
  path is dead. Gates reduce to elementwise functions of x @ W_g.
- The only graph op is the GCNConv aggregation with symmetric norm over
  A + I (edge_index + self loops, unit weights).

Sharding: nodes row-sharded across 8 cores (6272 per core, padded from
6250). Dense phase computed per shard in transposed layout; XW = H@convW
written node-major and AllGathered into a full gather table; aggregation
is destination-sorted: per 128-dest block, gather source rows via
indirect DMA, build selection matrices (is_equal vs iota, scaled by the
edge norm) and accumulate on the TensorEngine in PSUM. BatchNorm stats
are AllReduced. MaxPool over channel pairs is done by permuting the conv
output channels so pairs land at partitions (j, j+D/2).
"""
import numpy as np

import concourse.bacc as bacc
import concourse.bass as bass
import concourse.mybir as mybir
import concourse.tile as tile
from concourse import bass_utils

N = 50000
E = 800000
F_IN = 32
NCORES = 8
NS = 6272            # nodes per core (49 * 128)
NPAD = NS * NCORES   # 50176
NB = NS // 128       # 49 dest blocks per core
CHS = [(32, 128), (64, 64), (32, 32), (16, 16)]
EPS = 1e-5

fp32 = mybir.dt.float32
bf16 = mybir.dt.bfloat16
i32 = mybir.dt.int32

AGDT = fp32          # aggregation dtype (gather table + selection matrices)


def _perm(D):
    return np.concatenate([np.arange(0, D, 2), np.arange(1, D, 2)])


def _prep_graph(edge_index):
    """Destination-sorted, per-core, per-block wrapped edge arrays."""
    row = np.concatenate([edge_index[0], np.arange(N, dtype=np.int64)])
    col = np.concatenate([edge_index[1], np.arange(N, dtype=np.int64)])
    deg = np.bincount(col, minlength=N).astype(np.float64)
    dinv = 1.0 / np.sqrt(deg)
    gn = (dinv[row] * dinv[col]).astype(np.float32)

    order = np.argsort(col, kind="stable")
    row_s, col_s, gn_s = row[order], col[order], gn[order]
    nblk = NPAD // 128
    blk = col_s // 128
    cnt = np.bincount(blk, minlength=nblk)
    CB = int(np.ceil(cnt.max() / 128))
    starts = np.zeros(nblk + 1, np.int64)
    np.cumsum(cnt, out=starts[1:])

    gidx = np.zeros((NCORES, 128, NB * CB), np.int32)
    coll = np.zeros((NCORES, 128, NB * CB), np.float32)
    gnw = np.zeros((NCORES, 128, NB * CB), np.float32)
    mask = np.zeros((NCORES, 1, NS), np.float32)
    cap = CB * 128
    for r in range(NCORES):
        nreal = min(N - r * NS, NS)
        mask[r, 0, :nreal] = 1.0
        for b in range(NB):
            g = r * NB + b
            s, e = starts[g], starts[g + 1]
            n = e - s
            ridx = np.zeros(cap, np.int64)
            rcol = np.zeros(cap, np.float32)
            rgn = np.zeros(cap, np.float32)
            ridx[:n] = row_s[s:e]
            rcol[:n] = (col_s[s:e] % 128).astype(np.float32)
            rgn[:n] = gn_s[s:e]
            sl = slice(b * CB, (b + 1) * CB)
            gidx[r, :, sl] = ridx.reshape(CB, 128).T
            coll[r, :, sl] = rcol.reshape(CB, 128).T
            gnw[r, :, sl] = rgn.reshape(CB, 128).T
    return gidx, coll, gnw, mask, CB


def _prep_params(params):
    """Per-stage packed weights (channel-permuted on the conv side)."""
    out = {}
    for s in range(4):
        p = params[f"s{s + 1}"]
        cin, D = CHS[s]
        pm = _perm(D)
        f32 = lambda a: np.asarray(a, np.float32)
        wg = np.concatenate(
            [f32(p["W_i"]), f32(p["W_c"]), f32(p["W_o"])], axis=1)
        vec = np.stack(
            [
                f32(p["b_i"]) + f32(p["cb_i"]),
                f32(p["b_c"]) + f32(p["cb_c"]),
                f32(p["b_o"]) + f32(p["cb_o"]),
                f32(p["wc_o"]),
                f32(p["gamma"])[pm],
                f32(p["beta"])[pm],
            ],
            axis=1,
        )  # [D, 6]
        out[f"wg{s}"] = wg
        out[f"vec{s}"] = vec
        out[f"cw{s}"] = f32(p["conv_W"])[:, pm].copy()
        out[f"cb{s}"] = f32(p["conv_b"])[pm][None, :].copy()
    out["lin"] = np.asarray(params["lin_W"], np.float32)
    out["linb"] = np.asarray(params["lin_b"], np.float32)[None, :]
    return out


def build_nc(CB, agdt=AGDT, debug_outputs=False, ablate=()):
    nc = bacc.Bacc("TRN2", target_bir_lowering=False, debug=False,
                   num_devices=NCORES, name="gnn", num_swdge_queues=4)
    EI, EO = "ExternalInput", "ExternalOutput"
    xT = nc.dram_tensor("xT", [F_IN, NS], fp32, kind=EI)
    gidx = nc.dram_tensor("gidx", [128, NB * CB], i32, kind=EI)
    coll = nc.dram_tensor("coll", [128, NB * CB], fp32, kind=EI)
    gnw = nc.dram_tensor("gnw", [128, NB * CB],
                         fp32 if agdt == fp32 else bf16, kind=EI)
    maskI = nc.dram_tensor("mask", [1, NS], fp32, kind=EI)
    iota2 = nc.dram_tensor("iota2", [128, 128], fp32, kind=EI)
    wg_t, vec_t, cw_t, cb_t = [], [], [], []
    for s in range(4):
        cin, D = CHS[s]
        wg_t.append(nc.dram_tensor(f"wg{s}", [cin, 3 * D], fp32, kind=EI))
        vec_t.append(nc.dram_tensor(f"vec{s}", [D, 6], fp32, kind=EI))
        cw_t.append(nc.dram_tensor(f"cw{s}", [D, D], fp32, kind=EI))
        cb_t.append(nc.dram_tensor(f"cb{s}", [1, D], fp32, kind=EI))
    lin = nc.dram_tensor("lin", [8, 1], fp32, kind=EI)
    linb = nc.dram_tensor("linb", [1, 1], fp32, kind=EI)
    yout = nc.dram_tensor("y", [1, NS], fp32, kind=EO)
    dbg = {}
    if debug_outputs:
        for s in range(4):
            cin, D = CHS[s]
            dbg[f"xw{s}"] = nc.dram_tensor(f"dxw{s}", [NS, D], fp32, kind=EO)
            dbg[f"agg{s}"] = nc.dram_tensor(f"dagg{s}", [D, NS], fp32, kind=EO)
            dbg[f"xn{s}"] = nc.dram_tensor(f"dxn{s}", [D // 2, NS], fp32,
                                           kind=EO)

    with tile.TileContext(nc) as tc:
        with tc.tile_pool(name="io", bufs=1) as io, \
             tc.tile_pool(name="sb", bufs=3) as sb, \
             tc.tile_pool(name="den", bufs=2) as den, \
             tc.tile_pool(name="xn", bufs=2) as xn, \
             tc.tile_pool(name="psg", bufs=1, space="PSUM") as psg, \
             tc.tile_pool(name="psc", bufs=1, space="PSUM") as psc, \
             tc.tile_pool(name="psa", bufs=3, space="PSUM") as psa, \
             tc.tile_pool(name="dram", bufs=1, space="DRAM") as dram:

            # ---- constant loads ----
            x0 = xn.tile([F_IN, NS], fp32, tag="xcur", name="x0")
            nc.sync.dma_start(out=x0[:], in_=xT[:])
            gidx_sb = io.tile([128, NB * CB], i32)
            nc.sync.dma_start(out=gidx_sb[:], in_=gidx[:])
            coll_sb = io.tile([128, NB * CB], fp32)
            nc.sync.dma_start(out=coll_sb[:], in_=coll[:])
            gn_sb = io.tile([128, NB * CB], fp32 if agdt == fp32 else bf16)
            nc.sync.dma_start(out=gn_sb[:], in_=gnw[:])
            iota_sb = io.tile([128, 128], fp32)
            nc.sync.dma_start(out=iota_sb[:], in_=iota2[:])
            wg_sb, vec_sb, cw_sb, cb_sb = [], [], [], []
            for s in range(4):
                cin, D = CHS[s]
                w = io.tile([cin, 3 * D], fp32, name=f"wgs{s}")
                nc.sync.dma_start(out=w[:], in_=wg_t[s][:])
                wg_sb.append(w)
                v = io.tile([D, 6], fp32, name=f"vecs{s}")
                nc.sync.dma_start(out=v[:], in_=vec_t[s][:])
                vec_sb.append(v)
                cwt = io.tile([D, D], fp32, name=f"cws{s}")
                nc.sync.dma_start(out=cwt[:], in_=cw_t[s][:])
                cw_sb.append(cwt)
                cbt = io.tile([1, D], fp32, name=f"cbs{s}")
                nc.sync.dma_start(out=cbt[:], in_=cb_t[s][:])
                cb_sb.append(cbt)
            lin_sb = io.tile([8, 1], fp32)
            nc.sync.dma_start(out=lin_sb[:], in_=lin[:])
            linb_sb = io.tile([1, 1], fp32)
            nc.sync.dma_start(out=linb_sb[:], in_=linb[:])

            y_sh = io.tile([128, NS], fp32)        # post-relu agg output
            x_cur = x0
            Sig = mybir.ActivationFunctionType.Sigmoid
            Tanh = mybir.ActivationFunctionType.Tanh
            Relu = mybir.ActivationFunctionType.Relu
            Copy = mybir.ActivationFunctionType.Copy
            Sqrt = mybir.ActivationFunctionType.Sqrt
            AX = mybir.AxisListType.X

            for s in range(4):
                cin, D = CHS[s]
                vs = vec_sb[s]
                b_i, b_c, b_o = vs[:, 0:1], vs[:, 1:2], vs[:, 2:3]
                wco, gam, bet = vs[:, 3:4], vs[:, 4:5], vs[:, 5:6]
                xw_sh = dram.tile([NS, D], agdt, name=f"xwsh{s}")
                xw_full = dram.tile([NPAD, D], agdt, name=f"xwfull{s}",
                                    addr_space="Shared")
                # ---------- dense GCLSTM + conv ----------
                for j0 in range(0, NS, 512):
                    cw = min(512, NS - j0)
                    sl = slice(j0, j0 + cw)
                    pi = psg.tile([D, 512], fp32, space="PSUM", tag="pi")
                    pc = psg.tile([D, 512], fp32, space="PSUM", tag="pc")
                    po = psg.tile([D, 512], fp32, space="PSUM", tag="po")
                    nc.tensor.matmul(out=pi[:, :cw], lhsT=wg_sb[s][:, 0:D],
                                     rhs=x_cur[:, sl], start=True, stop=True)
                    nc.tensor.matmul(out=pc[:, :cw], lhsT=wg_sb[s][:, D:2 * D],
                                     rhs=x_cur[:, sl], start=True, stop=True)
                    nc.tensor.matmul(out=po[:, :cw], lhsT=wg_sb[s][:, 2 * D:],
                                     rhs=x_cur[:, sl], start=True, stop=True)
                    I_t = den.tile([D, 512], fp32, tag="It")
                    T_t = den.tile([D, 512], fp32, tag="Tt")
                    nc.scalar.activation(out=I_t[:, :cw], in_=pi[:, :cw],
                                         func=Sig, bias=b_i)
                    nc.scalar.activation(out=T_t[:, :cw], in_=pc[:, :cw],
                                         func=Tanh, bias=b_c)
                    C_t = den.tile([D, 512], fp32, tag="Ct")
                    nc.vector.tensor_mul(out=C_t[:, :cw], in0=I_t[:, :cw],
                                         in1=T_t[:, :cw])
                    t2 = den.tile([D, 512], fp32, tag="t2")
                    nc.vector.tensor_scalar(out=t2[:, :cw], in0=C_t[:, :cw],
                                            scalar1=wco, scalar2=None,
                                            op0=mybir.AluOpType.mult)
                    nc.vector.tensor_add(out=t2[:, :cw], in0=t2[:, :cw],
                                         in1=po[:, :cw])
                    O_t = den.tile([D, 512], fp32, tag="Ot")
                    nc.scalar.activation(out=O_t[:, :cw], in_=t2[:, :cw],
                                         func=Sig, bias=b_o)
                    tC = den.tile([D, 512], fp32, tag="tC")
                    nc.scalar.activation(out=tC[:, :cw], in_=C_t[:, :cw],
                                         func=Tanh)
                    H_t = den.tile([D, 512], fp32, tag="Ht")
                    nc.vector.tensor_mul(out=H_t[:, :cw], in0=O_t[:, :cw],
                                         in1=tC[:, :cw])
                    for k0 in range(0, cw, 128):
                        pxw = psc.tile([128, D], fp32, space="PSUM", tag="pxw")
                        nc.tensor.matmul(out=pxw[:], lhsT=H_t[:, k0:k0 + 128],
                                         rhs=cw_sb[s][:], start=True,
                                         stop=True)
                        ev = den.tile([128, D], agdt, tag="ev")
                        nc.scalar.activation(out=ev[:], in_=pxw[:], func=Copy)
                        nc.sync.dma_start(
                            out=xw_sh[j0 + k0:j0 + k0 + 128, :], in_=ev[:])
                if debug_outputs:
                    nc.gpsimd.dma_start(out=dbg[f"xw{s}"][:], in_=xw_sh[:])
                # ---------- allgather ----------
                nc.gpsimd.collective_compute(
                    "AllGather", mybir.AluOpType.bypass,
                    replica_groups=[list(range(NCORES))],
                    ins=[xw_sh.opt()], outs=[xw_full.opt()],
                )
                # ---------- aggregation ----------
                sums = den.tile([128, NB], fp32, tag="sums", bufs=1)
                sumq = den.tile([128, NB], fp32, tag="sumq", bufs=1)
                for b in range(NB):
                    G = sb.tile([128, CB * D], agdt, tag="G")
                    if "gather" in ablate:
                        nc.gpsimd.memset(G[:], 0)
                    for c in range(CB) if "gather" not in ablate else ():
                        gi = nc.gpsimd.indirect_dma_start(
                            out=G[:, c * D:(c + 1) * D], out_offset=None,
                            in_=xw_full[:],
                            in_offset=bass.IndirectOffsetOnAxis(
                                ap=gidx_sb[:, b * CB + c:b * CB + c + 1],
                                axis=0),
                        )
                        q = c % 4
                        gi.ins.queue = f"qPoolDynamic{q if q else ''}"

                    S = sb.tile([128, CB * 128], agdt, tag="S")
                    if "sbuild" in ablate:
                        nc.gpsimd.memset(S[:], 0)
                    if "sbuild" not in ablate:
                        S3 = S[:].rearrange("p (c j) -> p c j", c=CB)
                        c0 = coll_sb[:, b * CB:(b + 1) * CB][:, :, None] \
                            .to_broadcast([128, CB, 128])
                        i1 = iota_sb[:, None, :].to_broadcast([128, CB, 128])
                        nc.vector.tensor_tensor(out=S3, in0=c0, in1=i1,
                                                op=mybir.AluOpType.is_equal)
                        g1 = gn_sb[:, b * CB:(b + 1) * CB][:, :, None] \
                            .to_broadcast([128, CB, 128])
                        nc.vector.tensor_tensor(out=S3, in0=S3, in1=g1,
                                                op=mybir.AluOpType.mult)
                    pa = psa.tile([D, 128], fp32, space="PSUM", tag="pa")
                    for c in range(CB) if "mm" not in ablate else range(1):
                        nc.tensor.matmul(out=pa[:],
                                         lhsT=G[:, c * D:(c + 1) * D],
                                         rhs=S[:, c * 128:(c + 1) * 128],
                                         start=(c == 0), stop=False)
                    mb = den.tile([1, 128], fp32, tag="maskb")
                    nc.sync.dma_start(out=mb[:],
                                      in_=maskI[0:1, b * 128:(b + 1) * 128])
                    nc.tensor.matmul(out=pa[:], lhsT=cb_sb[s][:],
                                     rhs=mb[:], start=False, stop=True)
                    ysl = y_sh[0:D, b * 128:(b + 1) * 128]
                    nc.scalar.activation(out=ysl, in_=pa[:], func=Relu)
                    q = den.tile([128, 128], fp32, tag="ysq")
                    nc.vector.tensor_mul(out=q[0:D, :], in0=ysl, in1=ysl)
                    nc.vector.reduce_sum(out=sums[0:D, b:b + 1], in_=ysl,
                                         axis=AX)
                    nc.vector.reduce_sum(out=sumq[0:D, b:b + 1],
                                         in_=q[0:D, :], axis=AX)
                if debug_outputs:
                    nc.gpsimd.dma_start(out=dbg[f"agg{s}"][:],
                                        in_=y_sh[0:D, :])
                # ---------- batchnorm stats + allreduce ----------
                st = den.tile([128, 2], fp32, tag="stat", bufs=1)
                nc.gpsimd.memset(st[:], 0.0)
                nc.vector.reduce_sum(out=st[0:D, 0:1], in_=sums[0:D, :],
                                     axis=AX)
                nc.vector.reduce_sum(out=st[0:D, 1:2], in_=sumq[0:D, :],
                                     axis=AX)
                st_in = dram.tile([128, 2], fp32, name=f"stin{s}")
                st_out = dram.tile([128, 2], fp32, name=f"stout{s}",
                                   addr_space="Shared")
                nc.gpsimd.dma_start(out=st_in[:], in_=st[:])
                nc.gpsimd.collective_compute(
                    "AllReduce", mybir.AluOpType.add,
                    replica_groups=[list(range(NCORES))],
                    ins=[st_in.opt()], outs=[st_out.opt()],
                )
                stg = den.tile([128, 2], fp32, tag="statg", bufs=1)
                nc.sync.dma_start(out=stg[:], in_=st_out[:])
                mean = den.tile([128, 1], fp32, tag="mean", bufs=1)
                msq = den.tile([128, 1], fp32, tag="msq", bufs=1)
                nc.scalar.activation(out=mean[0:D, :], in_=stg[0:D, 0:1],
                                     func=Copy, scale=1.0 / N)
                nc.scalar.activation(out=msq[0:D, :], in_=stg[0:D, 1:2],
                                     func=Copy, scale=1.0 / N)
                var = den.tile([128, 1], fp32, tag="var", bufs=1)
                nc.vector.tensor_mul(out=var[0:D, :], in0=mean[0:D, :],
                                     in1=mean[0:D, :])
                nc.vector.tensor_sub(out=var[0:D, :], in0=msq[0:D, :],
                                     in1=var[0:D, :])
                nc.vector.tensor_scalar_add(var[0:D, :], var[0:D, :], EPS)
                rstd = den.tile([128, 1], fp32, tag="rstd", bufs=1)
                nc.scalar.activation(out=rstd[0:D, :], in_=var[0:D, :],
                                     func=Sqrt)
                nc.vector.reciprocal(out=rstd[0:D, :], in_=rstd[0:D, :])
                scl = den.tile([128, 1], fp32, tag="scl", bufs=1)
                nc.vector.tensor_mul(out=scl[0:D, :], in0=gam, in1=rstd[0:D, :])
                shf = den.tile([128, 1], fp32, tag="shf", bufs=1)
                nc.vector.tensor_mul(out=shf[0:D, :], in0=mean[0:D, :],
                                     in1=scl[0:D, :])
                nc.vector.tensor_sub(out=shf[0:D, :], in0=bet,
                                     in1=shf[0:D, :])
                # ---------- bn apply + maxpool ----------
                x_nx = xn.tile([D // 2, NS], fp32, tag="xcur", name=f"xnx{s}")
                for j0 in range(0, NS, 512):
                    cw = min(512, NS - j0)
                    sl = slice(j0, j0 + cw)
                    t = den.tile([128, 512], fp32, tag="bnap")
                    nc.vector.tensor_scalar(
                        out=t[0:D, :cw], in0=y_sh[0:D, sl],
                        scalar1=scl[0:D, :], scalar2=shf[0:D, :],
                        op0=mybir.AluOpType.mult, op1=mybir.AluOpType.add)
                    hi = den.tile([64, 512], fp32, tag="poolhi")
                    if (D // 2) % 32 == 0:
                        nc.scalar.activation(out=hi[0:D // 2, :cw],
                                             in_=t[D // 2:D, :cw], func=Copy)
                    else:
                        nc.sync.dma_start(out=hi[0:D // 2, :cw],
                                          in_=t[D // 2:D, :cw])
                    nc.vector.tensor_tensor(
                        out=x_nx[:, sl], in0=t[0:D // 2, :cw],
                        in1=hi[0:D // 2, :cw], op=mybir.AluOpType.max)
                if debug_outputs:
                    nc.gpsimd.dma_start(out=dbg[f"xn{s}"][:], in_=x_nx[:])
                x_cur = x_nx
            # ---------- final linear ----------
            for j0 in range(0, NS, 512):
                cw = min(512, NS - j0)
                pf = psc.tile([1, 512], fp32, space="PSUM", tag="pf")
                nc.tensor.matmul(out=pf[:, :cw], lhsT=lin_sb[:],
                                 rhs=x_cur[:, j0:j0 + cw], start=True,
                                 stop=True)
                ob = den.tile([1, 512], fp32, tag="ob")
                nc.vector.tensor_scalar_add(ob[:, :cw], pf[:, :cw],
                                            linb_sb[:, 0:1])
                nc.sync.dma_start(out=yout[0:1, j0:j0 + cw], in_=ob[0:1, :cw])
    nc.compile()
    return nc


def prep_inputs(x, edge_index, params, agdt=AGDT):
    x = np.asarray(x, np.float32)
    edge_index = np.asarray(edge_index)
    gidx, coll, gnw, mask, CB = _prep_graph(edge_index.astype(np.int64))
    pw = _prep_params(params)
    iota_np = np.broadcast_to(
        np.arange(128, dtype=np.float32), (128, 128)).copy()
    xpad = np.zeros((NPAD, F_IN), np.float32)
    xpad[:N] = x
    gn_dt = np.float32 if agdt == fp32 else mybir.dt.np(bf16)
    in_maps = []
    for r in range(NCORES):
        m = {
            "xT": np.ascontiguousarray(xpad[r * NS:(r + 1) * NS].T),
            "gidx": gidx[r],
            "coll": coll[r],
            "gnw": gnw[r].astype(gn_dt),
            "mask": mask[r],
            "iota2": iota_np,
        }
        m.update(pw)
        in_maps.append(m)
    return in_maps, CB


def kernel(x, edge_index, edge_weight, params):
    in_maps, CB = prep_inputs(x, edge_index, params)
    nc = build_nc(CB)
    res = bass_utils.run_bass_kernel_spmd(
        nc, in_maps, core_ids=list(range(NCORES)))
    parts = [res.results[r]["y"][0] for r in range(NCORES)]
    full = np.concatenate(parts)[:N]
    return full[:, None].astype(np.float32)


# revision 20
# speedup vs baseline: 4.6815x; 4.6815x over previous
"""Trainium2 Bass kernel for nn_CustomModel_30451318129113 (GCLSTM+GCNConv GNN).

Math notes (verified against the reference):
- The GCLSTM cell is called with H = C = 0, so every ChebConv sees an
  all-zero input and contributes only its bias => the Chebyshev/edge_weight
  path is dead. Gates reduce to elementwise functions of x @ W_g.
- The only graph op is the GCNConv aggregation with symmetric norm over
  A + I (edge_index + self loops, unit weights).

Sharding: nodes row-sharded across 8 cores (6272 per core, padded from
6250). Dense phase computed per shard in transposed layout; XW = H@convW
written node-major and AllGathered into a full gather table; aggregation
is destination-sorted: per 128-dest block, gather source rows via
indirect DMA, build selection matrices (is_equal vs iota, scaled by the
edge norm) and accumulate on the TensorEngine in PSUM. BatchNorm stats
are AllReduced. MaxPool over channel pairs is done by permuting the conv
output channels so pairs land at partitions (j, j+D/2).
"""
import numpy as np

import concourse.bacc as bacc
import concourse.bass as bass
import concourse.mybir as mybir
import concourse.tile as tile
from concourse import bass_utils

N = 50000
E = 800000
F_IN = 32
NCORES = 8
NS = 6272            # nodes per core (49 * 128)
NPAD = NS * NCORES   # 50176
NB = NS // 128       # 49 dest blocks per core
CHS = [(32, 128), (64, 64), (32, 32), (16, 16)]
EPS = 1e-5

fp32 = mybir.dt.float32
bf16 = mybir.dt.bfloat16
i32 = mybir.dt.int32

AGDT = fp32          # aggregation dtype (gather table + selection matrices)


def _perm(D):
    return np.concatenate([np.arange(0, D, 2), np.arange(1, D, 2)])


def _prep_graph(edge_index):
    """Destination-sorted, per-core, per-block wrapped edge arrays."""
    row = np.concatenate([edge_index[0], np.arange(N, dtype=np.int64)])
    col = np.concatenate([edge_index[1], np.arange(N, dtype=np.int64)])
    deg = np.bincount(col, minlength=N).astype(np.float64)
    dinv = 1.0 / np.sqrt(deg)
    gn = (dinv[row] * dinv[col]).astype(np.float32)

    HALF = NPAD // 2
    half = (row >= HALF).astype(np.int64)
    key = (col // 128) * 2 + half
    order = np.argsort(key, kind="stable")
    row_s, col_s, gn_s, key_s = row[order], col[order], gn[order], key[order]
    nkey = (NPAD // 128) * 2
    cnt = np.bincount(key_s, minlength=nkey)
    CBH = int(np.ceil(cnt.max() / 128))
    CB = 2 * CBH
    starts = np.zeros(nkey + 1, np.int64)
    np.cumsum(cnt, out=starts[1:])

    gidx = np.zeros((NCORES, 128, NB * CB * 8), np.int16)
    coll = np.zeros((NCORES, 128, NB * CB), np.float32)
    gnw = np.zeros((NCORES, 128, NB * CB), np.float32)
    mask = np.zeros((NCORES, 1, NS), np.float32)
    cap = CBH * 128
    p16 = np.arange(128) % 16
    for r in range(NCORES):
        nreal = min(N - r * NS, NS)
        mask[r, 0, :nreal] = 1.0
        for b in range(NB):
            for h in range(2):
                g = (r * NB + b) * 2 + h
                s, e = starts[g], starts[g + 1]
                n = e - s
                ridx = np.zeros(cap, np.int64)
                rcol = np.zeros(cap, np.float32)
                rgn = np.zeros(cap, np.float32)
                ridx[:n] = row_s[s:e] - h * HALF
                rcol[:n] = (col_s[s:e] % 128).astype(np.float32)
                rgn[:n] = gn_s[s:e]
                cc = b * CB + h * CBH
                coll[r, :, cc:cc + CBH] = rcol.reshape(CBH, 128).T
                gnw[r, :, cc:cc + CBH] = rgn.reshape(CBH, 128).T
                iw = ridx.reshape(CBH * 8, 16)[:, p16].T  # [128, CBH*8]
                jo = (b * 2 + h) * CBH * 8
                gidx[r, :, jo:jo + CBH * 8] = iw.astype(np.int16)
    return gidx, coll, gnw, mask, CB


def _prep_params(params):
    """Per-stage packed weights (channel-permuted on the conv side)."""
    out = {}
    for s in range(4):
        p = params[f"s{s + 1}"]
        cin, D = CHS[s]
        pm = _perm(D)
        f32 = lambda a: np.asarray(a, np.float32)
        wg = np.concatenate(
            [f32(p["W_i"]), f32(p["W_c"]), f32(p["W_o"])], axis=1)
        vec = np.stack(
            [
                f32(p["b_i"]) + f32(p["cb_i"]),
                f32(p["b_c"]) + f32(p["cb_c"]),
                f32(p["b_o"]) + f32(p["cb_o"]),
                f32(p["wc_o"]),
                f32(p["gamma"])[pm],
                f32(p["beta"])[pm],
            ],
            axis=1,
        )  # [D, 6]
        out[f"wg{s}"] = wg
        out[f"vec{s}"] = vec
        out[f"cw{s}"] = f32(p["conv_W"])[:, pm].copy()
        out[f"cb{s}"] = f32(p["conv_b"])[pm][None, :].copy()
    out["lin"] = np.asarray(params["lin_W"], np.float32)
    out["linb"] = np.asarray(params["lin_b"], np.float32)[None, :]
    return out


def build_nc(CB, agdt=AGDT, debug_outputs=False, ablate=()):
    nc = bacc.Bacc("TRN2", target_bir_lowering=False, debug=False,
                   num_devices=NCORES, name="gnn", num_swdge_queues=4)
    EI, EO = "ExternalInput", "ExternalOutput"
    xT = nc.dram_tensor("xT", [F_IN, NS], fp32, kind=EI)
    gidx = nc.dram_tensor("gidx", [128, NB * CB * 8], mybir.dt.int16, kind=EI)
    coll = nc.dram_tensor("coll", [128, NB * CB], fp32, kind=EI)
    gnw = nc.dram_tensor("gnw", [128, NB * CB],
                         fp32 if agdt == fp32 else bf16, kind=EI)
    maskI = nc.dram_tensor("mask", [1, NS], fp32, kind=EI)
    iota2 = nc.dram_tensor("iota2", [128, 128], fp32, kind=EI)
    wg_t, vec_t, cw_t, cb_t = [], [], [], []
    for s in range(4):
        cin, D = CHS[s]
        wg_t.append(nc.dram_tensor(f"wg{s}", [cin, 3 * D], fp32, kind=EI))
        vec_t.append(nc.dram_tensor(f"vec{s}", [D, 6], fp32, kind=EI))
        cw_t.append(nc.dram_tensor(f"cw{s}", [D, D], fp32, kind=EI))
        cb_t.append(nc.dram_tensor(f"cb{s}", [1, D], fp32, kind=EI))
    lin = nc.dram_tensor("lin", [8, 1], fp32, kind=EI)
    linb = nc.dram_tensor("linb", [1, 1], fp32, kind=EI)
    yout = nc.dram_tensor("y", [1, NS], fp32, kind=EO)
    dbg = {}
    if debug_outputs:
        for s in range(4):
            cin, D = CHS[s]
            dbg[f"xw{s}"] = nc.dram_tensor(f"dxw{s}", [NS, D], fp32, kind=EO)
            dbg[f"agg{s}"] = nc.dram_tensor(f"dagg{s}", [D, NS], fp32, kind=EO)
            dbg[f"xn{s}"] = nc.dram_tensor(f"dxn{s}", [D // 2, NS], fp32,
                                           kind=EO)

    with tile.TileContext(nc) as tc:
        with tc.tile_pool(name="io", bufs=1) as io, \
             tc.tile_pool(name="sb", bufs=3) as sb, \
             tc.tile_pool(name="den", bufs=2) as den, \
             tc.tile_pool(name="xn", bufs=2) as xn, \
             tc.tile_pool(name="psg", bufs=1, space="PSUM") as psg, \
             tc.tile_pool(name="psc", bufs=1, space="PSUM") as psc, \
             tc.tile_pool(name="psa", bufs=3, space="PSUM") as psa, \
             tc.tile_pool(name="dram", bufs=1, space="DRAM") as dram:

            # ---- constant loads ----
            x0 = xn.tile([F_IN, NS], fp32, tag="xcur", name="x0")
            nc.sync.dma_start(out=x0[:], in_=xT[:])
            gidx_sb = io.tile([128, NB * CB * 8], mybir.dt.int16)
            nc.sync.dma_start(out=gidx_sb[:], in_=gidx[:])
            coll_sb = io.tile([128, NB * CB], fp32)
            nc.sync.dma_start(out=coll_sb[:], in_=coll[:])
            gn_sb = io.tile([128, NB * CB], fp32 if agdt == fp32 else bf16)
            nc.sync.dma_start(out=gn_sb[:], in_=gnw[:])
            iota_sb = io.tile([128, 128], fp32)
            nc.sync.dma_start(out=iota_sb[:], in_=iota2[:])
            wg_sb, vec_sb, cw_sb, cb_sb = [], [], [], []
            for s in range(4):
                cin, D = CHS[s]
                w = io.tile([cin, 3 * D], fp32, name=f"wgs{s}")
                nc.sync.dma_start(out=w[:], in_=wg_t[s][:])
                wg_sb.append(w)
                v = io.tile([D, 6], fp32, name=f"vecs{s}")
                nc.sync.dma_start(out=v[:], in_=vec_t[s][:])
                vec_sb.append(v)
                cwt = io.tile([D, D], fp32, name=f"cws{s}")
                nc.sync.dma_start(out=cwt[:], in_=cw_t[s][:])
                cw_sb.append(cwt)
                cbt = io.tile([1, D], fp32, name=f"cbs{s}")
                nc.sync.dma_start(out=cbt[:], in_=cb_t[s][:])
                cb_sb.append(cbt)
            lin_sb = io.tile([8, 1], fp32)
            nc.sync.dma_start(out=lin_sb[:], in_=lin[:])
            linb_sb = io.tile([1, 1], fp32)
            nc.sync.dma_start(out=linb_sb[:], in_=linb[:])

            y_sh = io.tile([128, NS], fp32)        # post-relu agg output
            x_cur = x0
            Sig = mybir.ActivationFunctionType.Sigmoid
            Tanh = mybir.ActivationFunctionType.Tanh
            Relu = mybir.ActivationFunctionType.Relu
            Copy = mybir.ActivationFunctionType.Copy
            Sqrt = mybir.ActivationFunctionType.Sqrt
            AX = mybir.AxisListType.X

            for s in range(4):
                cin, D = CHS[s]
                vs = vec_sb[s]
                b_i, b_c, b_o = vs[:, 0:1], vs[:, 1:2], vs[:, 2:3]
                wco, gam, bet = vs[:, 3:4], vs[:, 4:5], vs[:, 5:6]
                Dg = max(D, 64)
                xw_sh = dram.tile([NS, Dg], agdt, name=f"xwsh{s}")
                xw_full = dram.tile([NPAD, Dg], agdt, name=f"xwfull{s}",
                                    addr_space="Shared")
                # ---------- dense GCLSTM + conv ----------
                for j0 in range(0, NS, 512):
                    cw = min(512, NS - j0)
                    sl = slice(j0, j0 + cw)
                    pi = psg.tile([D, 512], fp32, space="PSUM", tag="pi")
                    pc = psg.tile([D, 512], fp32, space="PSUM", tag="pc")
                    po = psg.tile([D, 512], fp32, space="PSUM", tag="po")
                    nc.tensor.matmul(out=pi[:, :cw], lhsT=wg_sb[s][:, 0:D],
                                     rhs=x_cur[:, sl], start=True, stop=True)
                    nc.tensor.matmul(out=pc[:, :cw], lhsT=wg_sb[s][:, D:2 * D],
                                     rhs=x_cur[:, sl], start=True, stop=True)
                    nc.tensor.matmul(out=po[:, :cw], lhsT=wg_sb[s][:, 2 * D:],
                                     rhs=x_cur[:, sl], start=True, stop=True)
                    I_t = den.tile([D, 512], fp32, tag="It")
                    T_t = den.tile([D, 512], fp32, tag="Tt")
                    nc.scalar.activation(out=I_t[:, :cw], in_=pi[:, :cw],
                                         func=Sig, bias=b_i)
                    nc.scalar.activation(out=T_t[:, :cw], in_=pc[:, :cw],
                                         func=Tanh, bias=b_c)
                    C_t = den.tile([D, 512], fp32, tag="Ct")
                    nc.vector.tensor_mul(out=C_t[:, :cw], in0=I_t[:, :cw],
                                         in1=T_t[:, :cw])
                    t2 = den.tile([D, 512], fp32, tag="t2")
                    nc.vector.tensor_scalar(out=t2[:, :cw], in0=C_t[:, :cw],
                                            scalar1=wco, scalar2=None,
                                            op0=mybir.AluOpType.mult)
                    nc.vector.tensor_add(out=t2[:, :cw], in0=t2[:, :cw],
                                         in1=po[:, :cw])
                    O_t = den.tile([D, 512], fp32, tag="Ot")
                    nc.scalar.activation(out=O_t[:, :cw], in_=t2[:, :cw],
                                         func=Sig, bias=b_o)
                    tC = den.tile([D, 512], fp32, tag="tC")
                    nc.scalar.activation(out=tC[:, :cw], in_=C_t[:, :cw],
                                         func=Tanh)
                    H_t = den.tile([D, 512], fp32, tag="Ht")
                    nc.vector.tensor_mul(out=H_t[:, :cw], in0=O_t[:, :cw],
                                         in1=tC[:, :cw])
                    for k0 in range(0, cw, 128):
                        pxw = psc.tile([128, D], fp32, space="PSUM", tag="pxw")
                        nc.tensor.matmul(out=pxw[:], lhsT=H_t[:, k0:k0 + 128],
                                         rhs=cw_sb[s][:], start=True,
                                         stop=True)
                        ev = den.tile([128, D], agdt, tag="ev")
                        nc.scalar.activation(out=ev[:], in_=pxw[:], func=Copy)
                        nc.sync.dma_start(
                            out=xw_sh[j0 + k0:j0 + k0 + 128, 0:D], in_=ev[:])
                if debug_outputs:
                    nc.gpsimd.dma_start(out=dbg[f"xw{s}"][:], in_=xw_sh[:, 0:D])
                # ---------- allgather ----------
                nc.gpsimd.collective_compute(
                    "AllGather", mybir.AluOpType.bypass,
                    replica_groups=[list(range(NCORES))],
                    ins=[xw_sh.opt()], outs=[xw_full.opt()],
                )
                # ---------- aggregation ----------
                sums = den.tile([128, NB], fp32, tag="sums", bufs=1)
                sumq = den.tile([128, NB], fp32, tag="sumq", bufs=1)
                CBH = CB // 2
                HALF = NPAD // 2
                for b in range(NB):
                    G = sb.tile([128, CB * Dg], agdt, tag="G")
                    for h in range(2):
                        jo = (b * 2 + h) * CBH * 8
                        nc.gpsimd.dma_gather(
                            out_ap=G[:, h * CBH * Dg:(h + 1) * CBH * Dg]
                            .rearrange("p (c d) -> p c d", d=Dg),
                            in_ap=xw_full[h * HALF:(h + 1) * HALF, :],
                            idxs_ap=gidx_sb[:, jo:jo + CBH * 8],
                            num_idxs=CBH * 128,
                            num_idxs_reg=CBH * 128,
                            elem_size=Dg,
                            single_packet=False,
                            queue_num=(b + h) % 4,
                        )

                    S = sb.tile([128, CB * 128], agdt, tag="S")
                    if "sbuild" in ablate:
                        nc.gpsimd.memset(S[:], 0)
                    if "sbuild" not in ablate:
                        S3 = S[:].rearrange("p (c j) -> p c j", c=CB)
                        c0 = coll_sb[:, b * CB:(b + 1) * CB][:, :, None] \
                            .to_broadcast([128, CB, 128])
                        i1 = iota_sb[:, None, :].to_broadcast([128, CB, 128])
                        nc.vector.tensor_tensor(out=S3, in0=c0, in1=i1,
                                                op=mybir.AluOpType.is_equal)
                        g1 = gn_sb[:, b * CB:(b + 1) * CB][:, :, None] \
                            .to_broadcast([128, CB, 128])
                        nc.vector.tensor_tensor(out=S3, in0=S3, in1=g1,
                                                op=mybir.AluOpType.mult)
                    pa = psa.tile([D, 128], fp32, space="PSUM", tag="pa")
                    for c in range(CB):
                        nc.tensor.matmul(out=pa[:],
                                         lhsT=G[:, c * Dg:c * Dg + D],
                                         rhs=S[:, c * 128:(c + 1) * 128],
                                         start=(c == 0), stop=False)
                    mb = den.tile([1, 128], fp32, tag="maskb")
                    nc.sync.dma_start(out=mb[:],
                                      in_=maskI[0:1, b * 128:(b + 1) * 128])
                    nc.tensor.matmul(out=pa[:], lhsT=cb_sb[s][:],
                                     rhs=mb[:], start=False, stop=True)
                    ysl = y_sh[0:D, b * 128:(b + 1) * 128]
                    nc.scalar.activation(out=ysl, in_=pa[:], func=Relu)
                    q = den.tile([128, 128], fp32, tag="ysq")
                    nc.vector.tensor_mul(out=q[0:D, :], in0=ysl, in1=ysl)
                    nc.vector.reduce_sum(out=sums[0:D, b:b + 1], in_=ysl,
                                         axis=AX)
                    nc.vector.reduce_sum(out=sumq[0:D, b:b + 1],
                                         in_=q[0:D, :], axis=AX)
                if debug_outputs:
                    nc.gpsimd.dma_start(out=dbg[f"agg{s}"][:],
                                        in_=y_sh[0:D, :])
                # ---------- batchnorm stats + allreduce ----------
                st = den.tile([128, 2], fp32, tag="stat", bufs=1)
                nc.gpsimd.memset(st[:], 0.0)
                nc.vector.reduce_sum(out=st[0:D, 0:1], in_=sums[0:D, :],
                                     axis=AX)
                nc.vector.reduce_sum(out=st[0:D, 1:2], in_=sumq[0:D, :],
                                     axis=AX)
                st_in = dram.tile([128, 2], fp32, name=f"stin{s}")
                st_out = dram.tile([128, 2], fp32, name=f"stout{s}",
                                   addr_space="Shared")
                nc.gpsimd.dma_start(out=st_in[:], in_=st[:])
                nc.gpsimd.collective_compute(
                    "AllReduce", mybir.AluOpType.add,
                    replica_groups=[list(range(NCORES))],
                    ins=[st_in.opt()], outs=[st_out.opt()],
                )
                stg = den.tile([128, 2], fp32, tag="statg", bufs=1)
                nc.sync.dma_start(out=stg[:], in_=st_out[:])
                mean = den.tile([128, 1], fp32, tag="mean", bufs=1)
                msq = den.tile([128, 1], fp32, tag="msq", bufs=1)
                nc.scalar.activation(out=mean[0:D, :], in_=stg[0:D, 0:1],
                                     func=Copy, scale=1.0 / N)
                nc.scalar.activation(out=msq[0:D, :], in_=stg[0:D, 1:2],
                                     func=Copy, scale=1.0 / N)
                var = den.tile([128, 1], fp32, tag="var", bufs=1)
                nc.vector.tensor_mul(out=var[0:D, :], in0=mean[0:D, :],
                                     in1=mean[0:D, :])
                nc.vector.tensor_sub(out=var[0:D, :], in0=msq[0:D, :],
                                     in1=var[0:D, :])
                nc.vector.tensor_scalar_add(var[0:D, :], var[0:D, :], EPS)
                rstd = den.tile([128, 1], fp32, tag="rstd", bufs=1)
                nc.scalar.activation(out=rstd[0:D, :], in_=var[0:D, :],
                                     func=Sqrt)
                nc.vector.reciprocal(out=rstd[0:D, :], in_=rstd[0:D, :])
                scl = den.tile([128, 1], fp32, tag="scl", bufs=1)
                nc.vector.tensor_mul(out=scl[0:D, :], in0=gam, in1=rstd[0:D, :])
                shf = den.tile([128, 1], fp32, tag="shf", bufs=1)
                nc.vector.tensor_mul(out=shf[0:D, :], in0=mean[0:D, :],
                                     in1=scl[0:D, :])
                nc.vector.tensor_sub(out=shf[0:D, :], in0=bet,
                                     in1=shf[0:D, :])
                # ---------- bn apply + maxpool ----------
                x_nx = xn.tile([D // 2, NS], fp32, tag="xcur", name=f"xnx{s}")
                for j0 in range(0, NS, 512):
                    cw = min(512, NS - j0)
                    sl = slice(j0, j0 + cw)
                    t = den.tile([128, 512], fp32, tag="bnap")
                    nc.vector.tensor_scalar(
                        out=t[0:D, :cw], in0=y_sh[0:D, sl],
                        scalar1=scl[0:D, :], scalar2=shf[0:D, :],
                        op0=mybir.AluOpType.mult, op1=mybir.AluOpType.add)
                    hi = den.tile([64, 512], fp32, tag="poolhi")
                    if (D // 2) % 32 == 0:
                        nc.scalar.activation(out=hi[0:D // 2, :cw],
                                             in_=t[D // 2:D, :cw], func=Copy)
                    else:
                        nc.sync.dma_start(out=hi[0:D // 2, :cw],
                                          in_=t[D // 2:D, :cw])
                    nc.vector.tensor_tensor(
                        out=x_nx[:, sl], in0=t[0:D // 2, :cw],
                        in1=hi[0:D // 2, :cw], op=mybir.AluOpType.max)
                if debug_outputs:
                    nc.gpsimd.dma_start(out=dbg[f"xn{s}"][:], in_=x_nx[:])
                x_cur = x_nx
            # ---------- final linear ----------
            for j0 in range(0, NS, 512):
                cw = min(512, NS - j0)
                pf = psc.tile([1, 512], fp32, space="PSUM", tag="pf")
                nc.tensor.matmul(out=pf[:, :cw], lhsT=lin_sb[:],
                                 rhs=x_cur[:, j0:j0 + cw], start=True,
                                 stop=True)
                ob = den.tile([1, 512], fp32, tag="ob")
                nc.vector.tensor_scalar_add(ob[:, :cw], pf[:, :cw],
                                            linb_sb[:, 0:1])
                nc.sync.dma_start(out=yout[0:1, j0:j0 + cw], in_=ob[0:1, :cw])
    nc.compile()
    return nc


def prep_inputs(x, edge_index, params, agdt=AGDT):
    x = np.asarray(x, np.float32)
    edge_index = np.asarray(edge_index)
    gidx, coll, gnw, mask, CB = _prep_graph(edge_index.astype(np.int64))
    pw = _prep_params(params)
    iota_np = np.broadcast_to(
        np.arange(128, dtype=np.float32), (128, 128)).copy()
    xpad = np.zeros((NPAD, F_IN), np.float32)
    xpad[:N] = x
    gn_dt = np.float32 if agdt == fp32 else mybir.dt.np(bf16)
    in_maps = []
    for r in range(NCORES):
        m = {
            "xT": np.ascontiguousarray(xpad[r * NS:(r + 1) * NS].T),
            "gidx": gidx[r],
            "coll": coll[r],
            "gnw": gnw[r].astype(gn_dt),
            "mask": mask[r],
            "iota2": iota_np,
        }
        m.update(pw)
        in_maps.append(m)
    return in_maps, CB


def kernel(x, edge_index, edge_weight, params):
    in_maps, CB = prep_inputs(x, edge_index, params)
    nc = build_nc(CB)
    res = bass_utils.run_bass_kernel_spmd(
        nc, in_maps, core_ids=list(range(NCORES)))
    parts = [res.results[r]["y"][0] for r in range(NCORES)]
    full = np.concatenate(parts)[:N]
    return full[:, None].astype(np.float32)
